# revision 15
# baseline (speedup 1.0000x reference)
"""Trainium2 Bass kernel for nn_Alignment loss (CORAL-style alignment loss).

Strategy (hardcoded for B=64, hat_L=8, N=16, d=32, 8 cores):
  - Shard over hat_L: core i handles layer t=i (SPMD, per-core input shards).
  - All covariance Frobenius terms use the Gram trick:
      ||Xc^T Xc - Yc^T Yc||_F^2 = ||Xc Xc^T||^2 - 2||Xc Yc^T||^2 + ||Yc Yc^T||^2
    so the device only materializes 64x64 batch Grams, never feature covs.
  - The batch Gram is computed on RAW (uncentered) data in float32r (full
    fp32 result); the exact rank-1 centering correction is applied on host
    in float64 from the raw inputs.  L_exo Grams are sums of per-t Grams.
  - The L_sfa tail (centering + transpose + per-node covariances) runs in
    bf16 (its final-loss contribution is ~1%, so bf16 error is ~1e-5 there);
    the 16x16 covariance inner products are then done on host in float64
    from the shipped bf16 C matrices.
  - E variance statistics are computed with PE ones-matmuls on batch-major
    data (keeps the DVE free for the critical centering chain).
  - Inputs are host-packed into the exact SBUF images (contiguous 1-2KB
    per-partition runs -> minimal DMA descriptor cost), one DMA per queue.
  - Device outputs per core: raw 2x2 block Gram [128,128] f32, the bf16
    per-node covariances [32, 2*16*32], and E-sum/E-sumsq [128,8] f32.
"""

import numpy as np

import concourse.bass as bass
import concourse.tile as tile
from concourse import mybir
from concourse.bass_utils import run_bass_kernel_spmd
from concourse.masks import make_identity

B = 64
T = 8
N = 16
D = 32
FW = N * D          # 512 flattened per-layer features
KCH = FW // 128     # 4 feature chunks of 128
ECH = (N * N) // 128  # 2 chunks for E features (256)
F32 = mybir.dt.float32
F32R = mybir.dt.float32r
BF16 = mybir.dt.bfloat16

_BUILT = None


def _r(ap):
    return ap.bitcast(F32R)


def _build():
    nc = bass.Bass()
    zlo = nc.dram_tensor("zlo", [64, KCH * 2 * B], F32, kind="ExternalInput")
    zhi = nc.dram_tensor("zhi", [64, KCH * 2 * B], F32, kind="ExternalInput")
    ee = nc.dram_tensor("ee", [B, 2 * N * N], F32, kind="ExternalInput")
    out_g = nc.dram_tensor("out_g", [128, 128], F32, kind="ExternalOutput")
    out_c = nc.dram_tensor("out_c", [32, 2 * N * D], BF16,
                           kind="ExternalOutput")
    out_e = nc.dram_tensor("out_e", [128, 2, ECH * 2], F32,
                           kind="ExternalOutput")

    with tile.TileContext(nc) as tc:
        with tc.tile_pool(name="sb", bufs=1) as sb, \
             tc.tile_pool(name="ps1", bufs=1, space="PSUM") as ps1:
            # ---- loads: one packed image per DMA queue -------------------
            Zb = sb.tile([128, KCH, 2, B], F32)   # interleaved [Zs_k|Zt_k]
            Ebm = sb.tile([B, 2, N * N], F32)     # batch-major E
            nc.sync.dma_start(
                out=Zb[0:64, :, :, :],
                in_=zlo[:].rearrange("p (k s b) -> p k s b", s=2, b=B))
            nc.scalar.dma_start(
                out=Zb[64:128, :, :, :],
                in_=zhi[:].rearrange("p (k s b) -> p k s b", s=2, b=B))
            nc.gpsimd.dma_start(out=Ebm[:, :, :],
                                in_=ee[:].rearrange("p (s f) -> p s f", s=2))

            identity = sb.tile([128, 128], BF16)
            make_identity(nc, identity)
            ones = sb.tile([B, 1], F32)
            nc.gpsimd.memset(ones[:, :], 1.0)
            # warm the ACT table for Copy while DMAs are in flight
            warm = sb.tile([1, 1], F32)
            nc.vector.memset(warm[:, :], 0.0)
            nc.scalar.copy(out=warm[:, :], in_=warm[:, :])

            # ---- center Z over batch -> bf16 (fused ops) -----------------
            zsums = sb.tile([128, 2, KCH], F32)
            Zc = sb.tile([128, KCH, 2, B], BF16)
            for s in range(2):
                nc.vector.reduce_sum(out=zsums[:, s, :], in_=Zb[:, :, s, :],
                                     axis=mybir.AxisListType.X)
            for s in range(2):
                sums_b = zsums[:, s, :].broadcast_to([128, KCH, B])
                eng = nc.vector if s == 0 else nc.gpsimd
                eng.scalar_tensor_tensor(
                    out=Zc[:, :, s, :], in0=sums_b, scalar=-1.0 / B,
                    in1=Zb[:, :, s, :], op0=mybir.AluOpType.mult,
                    op1=mybir.AluOpType.add)

            # ---- raw 2x2 block batch Gram [128,128] (f32r, exact) --------
            gpsum = ps1.tile([128, 128], F32)
            for k in range(KCH):
                blk = _r(Zb[:, k, :, :].rearrange("p s b -> p (s b)"))
                nc.tensor.matmul(gpsum[:, :], blk, blk,
                                 start=(k == 0), stop=(k == KCH - 1))
            Gsb = sb.tile([128, 128], F32)

            # ---- transpose centered Z (bf16) to batch-major --------------
            # Zbm rows: 0-63 = Zsc [64, 512], 64-127 = Ztc [64, 512]
            Zbm = sb.tile([128, KCH, 128], BF16)
            for half in range(2):
                tp = ps1.tile([128, 2, 128], BF16, tag=f"tp{half}")
                for i in range(2):
                    k = half * 2 + i
                    blk = Zc[:, k, :, :].rearrange("p s b -> p (s b)")
                    nc.tensor.transpose(tp[:, i, :], blk, identity[:, :])
                dst = Zbm[:, 2 * half:2 * half + 2, :]
                if half == 0:
                    nc.vector.tensor_copy(out=dst, in_=tp[:, :, :])
                else:
                    nc.scalar.copy(out=dst, in_=tp[:, :, :])

            # ---- per-node covariances C[n] = Zc_n^T Zc_n [32,32] ---------
            cst_ps = ps1.tile([32, 2, N, D], F32)
            STcat = sb.tile([32, 2, N, D], BF16)
            for src in range(2):
                lo, hi = (0, B) if src == 0 else (B, 128)
                for n in range(N):
                    k, c0 = divmod(n * D, 128)
                    lhs = Zbm[lo:hi, k, c0:c0 + D]
                    nc.tensor.matmul(cst_ps[:, src, n, :], lhs, lhs,
                                     start=True, stop=True)
                # per-source copy (distinct PSUM banks -> concurrent)
                if src == 0:
                    nc.vector.tensor_copy(out=STcat[:, src, :, :],
                                          in_=cst_ps[:, src, :, :])
                else:
                    nc.scalar.copy(out=STcat[:, src, :, :],
                                   in_=cst_ps[:, src, :, :])
            nc.sync.dma_start(
                out=out_c[:, :],
                in_=STcat[:, :, :, :].rearrange("p s n b -> p (s n b)"))

            nc.vector.tensor_copy(out=Gsb[:, :], in_=gpsum[:, :])
            nc.sync.dma_start(out=out_g[:, :], in_=Gsb[:, :])

            # ---- E sums / sumsq via PE ones-matmuls (off the DVE) --------
            Esq = sb.tile([B, 2, N * N], F32)
            nc.gpsimd.tensor_mul(Esq[:, :, :], Ebm[:, :, :], Ebm[:, :, :])
            epsum = ps1.tile([128, 2, ECH * 2], F32)
            for s in range(2):
                for c in range(ECH):
                    lhs = _r(Ebm[:, s, 128 * c:128 * (c + 1)])
                    nc.tensor.matmul(epsum[:, 0, 2 * c + s:2 * c + s + 1], lhs,
                                     _r(ones[:, :]), start=True, stop=True)
                    lhsq = _r(Esq[:, s, 128 * c:128 * (c + 1)])
                    nc.tensor.matmul(epsum[:, 1, 2 * c + s:2 * c + s + 1], lhsq,
                                     _r(ones[:, :]), start=True, stop=True)
            ES = sb.tile([128, 2, ECH * 2], F32)
            nc.scalar.copy(out=ES[:, :, :], in_=epsum[:, :, :])
            nc.gpsimd.dma_start(out=out_e[:, :, :], in_=ES[:, :, :])

    return nc


def _get_nc():
    global _BUILT
    if _BUILT is None:
        _BUILT = _build()
    return _BUILT


def _prep_in_maps(Z_s, E_s, Z_t, E_t):
    in_maps = []
    for t in range(T):
        # Zb image: [128 p, k, s, b] = Z_src[b, 128k+p], split by partition
        zzi = np.empty((128, KCH, 2, B), np.float32)
        zzi[:, :, 0, :] = Z_s[:, t].reshape(B, KCH, 128).transpose(2, 1, 0)
        zzi[:, :, 1, :] = Z_t[:, t].reshape(B, KCH, 128).transpose(2, 1, 0)
        # E image: batch-major [B, 2, 256]
        eei = np.empty((B, 2, N * N), np.float32)
        eei[:, 0, :] = E_s[:, t].reshape(B, N * N)
        eei[:, 1, :] = E_t[:, t].reshape(B, N * N)
        in_maps.append({
            "zlo": np.ascontiguousarray(zzi[0:64].reshape(64, KCH * 2 * B)),
            "zhi": np.ascontiguousarray(zzi[64:128].reshape(64, KCH * 2 * B)),
            "ee": np.ascontiguousarray(eei.reshape(B, 2 * N * N)),
        })
    return in_maps


def _combine(results, Z_s, Z_t):
    """Host-side (float64) combine of per-core partial reductions."""
    LAM = 0.1
    EPS = 1e-8
    Bm1 = B - 1

    Gss_sum = np.zeros((B, B), np.float64)
    Gst_sum = np.zeros((B, B), np.float64)
    Gtt_sum = np.zeros((B, B), np.float64)
    W = np.zeros(T, np.float64)
    L_sca = np.zeros(T, np.float64)
    L_sfa = np.zeros(T, np.float64)

    for t in range(T):
        r = results[t]
        g = r["out_g"].astype(np.float64).reshape(128, 128)
        # exact rank-1 centering corrections from the raw inputs
        Xs = Z_s[:, t].reshape(B, FW).astype(np.float64)
        Xt = Z_t[:, t].reshape(B, FW).astype(np.float64)
        mus, mut = Xs.mean(0), Xt.mean(0)
        Gss = g[:B, :B] - np.add.outer(Xs @ mus, Xs @ mus) + (mus @ mus)
        Gst = g[:B, B:] - np.add.outer(Xs @ mut, Xt @ mus) + (mus @ mut)
        Gtt = g[B:, B:] - np.add.outer(Xt @ mut, Xt @ mut) + (mut @ mut)
        Gss_sum += Gss
        Gst_sum += Gst
        Gtt_sum += Gtt
        num = (Gss * Gss).sum() - 2.0 * (Gst * Gst).sum() + (Gtt * Gtt).sum()
        W[t] = num / (Bm1 * Bm1 * 4.0 * FW * FW)

        # C matrices: out_c[a, (src, n, b)] = C_src[n, a, b] (bf16)
        c = r["out_c"].astype(np.float64).reshape(32, 2, N, D)
        Cs = c[:, 0].transpose(1, 0, 2) / Bm1   # [n, a, b]
        Ct = c[:, 1].transpose(1, 0, 2) / Bm1
        ss = np.einsum("nab,nab->n", Cs, Cs)
        tt = np.einsum("nab,nab->n", Ct, Ct)
        st = np.einsum("nab,jab->nj", Cs, Ct)
        Dm = (ss[:, None] + tt[None, :] - 2.0 * st) / (4.0 * D * D)
        pos = np.diag(Dm)
        neg = Dm.sum(axis=1) - pos
        L_sfa[t] = np.mean(np.log(np.exp(pos) + neg + EPS) - pos)

        e = r["out_e"].astype(np.float64).reshape(128, 2, ECH * 2)
        sums = e[:, 0, :].reshape(128, ECH, 2)
        sumsq = e[:, 1, :].reshape(128, ECH, 2)
        var = (sumsq - sums * sums / B) / Bm1
        dv = var[:, :, 0] - var[:, :, 1]
        L_sca[t] = np.mean(dv * dv) / 4.0

    fexo = T * FW
    num = ((Gss_sum * Gss_sum).sum() - 2.0 * (Gst_sum * Gst_sum).sum()
           + (Gtt_sum * Gtt_sum).sum())
    L_exo = num / (Bm1 * Bm1 * 4.0 * fexo * fexo)
    L_iendo = float((W * (LAM * L_sca + LAM * L_sfa)).sum())
    return np.float32(L_exo + L_iendo / T)


def _run(Z_s, E_s, Z_t, E_t, trace=False, **kw):
    nc = _get_nc()
    in_maps = _prep_in_maps(Z_s, E_s, Z_t, E_t)
    res = run_bass_kernel_spmd(nc, in_maps, core_ids=list(range(T)),
                               trace=trace, **kw)
    return _combine(res.results, Z_s, Z_t), res


def kernel(Z_s, E_s, Z_t, E_t):
    out, _ = _run(Z_s, E_s, Z_t, E_t)
    return out


# revision 18
# speedup vs baseline: 1.0322x; 1.0322x over previous
"""Trainium2 Bass kernel for nn_Alignment loss (CORAL-style alignment loss).

Strategy (hardcoded for B=64, hat_L=8, N=16, d=32, 8 cores):
  - Shard over hat_L: core i handles layer t=i (SPMD, per-core input shards).
  - All covariance Frobenius terms use the Gram trick:
      ||Xc^T Xc - Yc^T Yc||_F^2 = ||Xc Xc^T||^2 - 2||Xc Yc^T||^2 + ||Yc Yc^T||^2
    so the device only materializes 64x64 batch Grams, never feature covs.
  - The batch Gram is computed on RAW (uncentered) data in exact fp32; the
    rank-1 centering correction is applied on host in float64 from the raw
    inputs.  L_exo Grams are sums of per-t Grams (feature blocks).
  - The L_sfa tail (centering + transpose + per-node covariances) runs in
    bf16: its final-loss contribution is ~1%, so bf16 error is ~1e-5 on the
    output.  The 16x16 covariance inner products are done on host in
    float64 from the shipped bf16 C matrices.
  - E variance statistics use PE ones-matmuls on batch-major data; the
    ones column is embedded in the input/scratch images so every PE matmul
    carries at most one semaphore wait (hardware limit).
  - Inputs are host-packed into exact SBUF images (contiguous per-partition
    runs -> minimal DMA descriptor cost), one DMA per queue.
  - Device outputs per core: raw 2x2 block Gram [128,128] f32, bf16
    per-node covariances [32, 2*16*32], and E-sum/E-sumsq [128,8] f32.
"""

import numpy as np

import concourse.bass as bass
import concourse.tile as tile
from concourse import mybir
from concourse.bass_utils import run_bass_kernel_spmd

B = 64
T = 8
N = 16
D = 32
FW = N * D          # 512 flattened per-layer features
KCH = FW // 128     # 4 feature chunks of 128
ECH = (N * N) // 128  # 2 chunks for E features (256)
F32 = mybir.dt.float32
BF16 = mybir.dt.bfloat16

_BUILT = None


def _build():
    nc = bass.Bass()
    # za: chunks 0-1, zb: chunks 2-3; each [128, (k2, s, b)] image
    za = nc.dram_tensor("za", [128, 2 * 2 * B], F32, kind="ExternalInput")
    zb = nc.dram_tensor("zb", [128, 2 * 2 * B], F32, kind="ExternalInput")
    ee = nc.dram_tensor("ee", [B, 2 * N * N + 1], F32, kind="ExternalInput")
    out_g = nc.dram_tensor("out_g", [128, 128], F32, kind="ExternalOutput")
    out_c = nc.dram_tensor("out_c", [32, 2 * N * D], BF16,
                           kind="ExternalOutput")
    out_e = nc.dram_tensor("out_e", [128, 2, ECH * 2], F32,
                           kind="ExternalOutput")

    with tile.TileContext(nc) as tc:
        with tc.tile_pool(name="sb", bufs=1) as sb, \
             tc.tile_pool(name="ps1", bufs=1, space="PSUM") as ps1:
            # ---- loads: one packed image per DMA queue -------------------
            Zb = sb.tile([128, KCH, 2, B], F32)   # interleaved [Zs_k|Zt_k]
            Ebm = sb.tile([B, 2 * N * N + 1], F32)  # batch-major E + ones
            nc.sync.dma_start(
                out=Zb[:, 0:2, :, :],
                in_=za[:].rearrange("p (k s b) -> p k s b", s=2, b=B))
            nc.scalar.dma_start(
                out=Zb[:, 2:4, :, :],
                in_=zb[:].rearrange("p (k s b) -> p k s b", s=2, b=B))
            nc.gpsimd.dma_start(out=Ebm[:, :], in_=ee[:])

            # identity built on Pool, then fenced through the DVE so the
            # transposes wait on a single (DVE) semaphore
            identity0 = sb.tile([128, 128], BF16)
            nc.gpsimd.memset(identity0[:, :], 0.0)
            nc.gpsimd.affine_select(
                out=identity0[:, :], in_=identity0[:, :],
                compare_op=mybir.AluOpType.not_equal, fill=1.0,
                base=0, pattern=[[-1, 128]], channel_multiplier=1)
            identity = sb.tile([128, 128], BF16)
            nc.vector.tensor_copy(out=identity[:, :], in_=identity0[:, :])
            # warm the ACT table for Copy while DMAs are in flight
            warm = sb.tile([1, 1], F32)
            nc.vector.memset(warm[:, :], 0.0)
            nc.scalar.copy(out=warm[:, :], in_=warm[:, :])

            # ---- center Z over batch -> bf16, per chunk-pair on DVE ------
            zsums = sb.tile([128, 2, 2, 2], F32)  # [p, pair, s, k2]
            Zc = sb.tile([128, KCH, 2, B], BF16)
            for pair in range(2):
                ks = slice(2 * pair, 2 * pair + 2)
                for s in range(2):
                    nc.vector.reduce_sum(out=zsums[:, pair, s, :],
                                         in_=Zb[:, ks, s, :],
                                         axis=mybir.AxisListType.X)
                    sums_b = zsums[:, pair, s, :].broadcast_to([128, 2, B])
                    nc.vector.scalar_tensor_tensor(
                        out=Zc[:, ks, s, :], in0=sums_b, scalar=-1.0 / B,
                        in1=Zb[:, ks, s, :], op0=mybir.AluOpType.mult,
                        op1=mybir.AluOpType.add)

            # ---- raw 2x2 block batch Gram [128,128] (fp32, exact) --------
            gpsum = ps1.tile([128, 128], F32)
            for k in range(KCH):
                blk = Zb[:, k, :, :].rearrange("p s b -> p (s b)")
                nc.tensor.matmul(gpsum[:, :], blk, blk,
                                 start=(k == 0), stop=(k == KCH - 1))
            Gsb = sb.tile([128, 128], F32)

            # ---- transpose centered Z (bf16) to batch-major --------------
            # Zbm rows: 0-63 = Zsc [64, 512], 64-127 = Ztc [64, 512]
            Zbm = sb.tile([128, KCH, 128], BF16)
            for half in range(2):
                tp = ps1.tile([128, 2, 128], BF16, tag=f"tp{half}")
                for i in range(2):
                    k = half * 2 + i
                    blk = Zc[:, k, :, :].rearrange("p s b -> p (s b)")
                    nc.tensor.transpose(tp[:, i, :], blk, identity[:, :])
                dst = Zbm[:, 2 * half:2 * half + 2, :]
                if half == 0:
                    nc.vector.tensor_copy(out=dst, in_=tp[:, :, :])
                else:
                    nc.scalar.copy(out=dst, in_=tp[:, :, :])

            # ---- per-node covariances C[n] = Zc_n^T Zc_n [32,32] ---------
            cst_ps = ps1.tile([32, 2, N, D], F32)
            STcat = sb.tile([32, 2, N, D], BF16)
            for src in range(2):
                lo, hi = (0, B) if src == 0 else (B, 128)
                for n in range(N):
                    k, c0 = divmod(n * D, 128)
                    lhs = Zbm[lo:hi, k, c0:c0 + D]
                    nc.tensor.matmul(cst_ps[:, src, n, :], lhs, lhs,
                                     start=True, stop=True)
                # per-source copy (distinct PSUM banks -> concurrent)
                if src == 0:
                    nc.vector.tensor_copy(out=STcat[:, src, :, :],
                                          in_=cst_ps[:, src, :, :])
                else:
                    nc.scalar.copy(out=STcat[:, src, :, :],
                                   in_=cst_ps[:, src, :, :])
            nc.sync.dma_start(
                out=out_c[:, 0:N * D],
                in_=STcat[:, 0, :, :].rearrange("p n b -> p (n b)"))
            nc.sync.dma_start(
                out=out_c[:, N * D:],
                in_=STcat[:, 1, :, :].rearrange("p n b -> p (n b)"))

            nc.vector.tensor_copy(out=Gsb[:, :], in_=gpsum[:, :])
            nc.sync.dma_start(out=out_g[:, :], in_=Gsb[:, :])

            # ---- E sums / sumsq via PE ones-matmuls (off the DVE) --------
            # Esq carries its own ones column so the sumsq matmuls wait on
            # the Pool sem only; the sums matmuls wait on the ee DMA only.
            Esq = sb.tile([B, 2 * N * N + 1], F32)
            nc.gpsimd.tensor_mul(Esq[:, 0:2 * N * N],
                                 Ebm[:, 0:2 * N * N], Ebm[:, 0:2 * N * N])
            nc.gpsimd.memset(Esq[:, 2 * N * N:], 1.0)
            epsum = ps1.tile([128, 2, ECH * 2], F32)
            ev = Ebm[:, 0:2 * N * N].rearrange("p (s f) -> p s f", s=2)
            qv = Esq[:, 0:2 * N * N].rearrange("p (s f) -> p s f", s=2)
            for s in range(2):
                for c in range(ECH):
                    nc.tensor.matmul(
                        epsum[:, 0, 2 * c + s:2 * c + s + 1],
                        ev[:, s, 128 * c:128 * (c + 1)],
                        Ebm[:, 2 * N * N:], start=True, stop=True)
                    nc.tensor.matmul(
                        epsum[:, 1, 2 * c + s:2 * c + s + 1],
                        qv[:, s, 128 * c:128 * (c + 1)],
                        Esq[:, 2 * N * N:], start=True, stop=True)
            ES = sb.tile([128, 2, ECH * 2], F32)
            nc.scalar.copy(out=ES[:, :, :], in_=epsum[:, :, :])
            nc.gpsimd.dma_start(out=out_e[:, :, :], in_=ES[:, :, :])

    return nc


def _get_nc():
    global _BUILT
    if _BUILT is None:
        _BUILT = _build()
    return _BUILT


def _prep_in_maps(Z_s, E_s, Z_t, E_t):
    in_maps = []
    for t in range(T):
        # Zb image: [128 p, k, s, b] = Z_src[b, 128k+p], split by chunk pair
        zzi = np.empty((128, KCH, 2, B), np.float32)
        zzi[:, :, 0, :] = Z_s[:, t].reshape(B, KCH, 128).transpose(2, 1, 0)
        zzi[:, :, 1, :] = Z_t[:, t].reshape(B, KCH, 128).transpose(2, 1, 0)
        # E image: batch-major [B, 2*256], plus a trailing ones column
        eei = np.empty((B, 2 * N * N + 1), np.float32)
        eei[:, 0:N * N] = E_s[:, t].reshape(B, N * N)
        eei[:, N * N:2 * N * N] = E_t[:, t].reshape(B, N * N)
        eei[:, 2 * N * N] = 1.0
        in_maps.append({
            "za": np.ascontiguousarray(zzi[:, 0:2].reshape(128, 2 * 2 * B)),
            "zb": np.ascontiguousarray(zzi[:, 2:4].reshape(128, 2 * 2 * B)),
            "ee": np.ascontiguousarray(eei),
        })
    return in_maps


def _combine(results, Z_s, Z_t):
    """Host-side (float64) combine of per-core partial reductions."""
    LAM = 0.1
    EPS = 1e-8
    Bm1 = B - 1

    Gss_sum = np.zeros((B, B), np.float64)
    Gst_sum = np.zeros((B, B), np.float64)
    Gtt_sum = np.zeros((B, B), np.float64)
    W = np.zeros(T, np.float64)
    L_sca = np.zeros(T, np.float64)
    L_sfa = np.zeros(T, np.float64)

    for t in range(T):
        r = results[t]
        g = r["out_g"].astype(np.float64).reshape(128, 128)
        # exact rank-1 centering corrections from the raw inputs
        Xs = Z_s[:, t].reshape(B, FW).astype(np.float64)
        Xt = Z_t[:, t].reshape(B, FW).astype(np.float64)
        mus, mut = Xs.mean(0), Xt.mean(0)
        Gss = g[:B, :B] - np.add.outer(Xs @ mus, Xs @ mus) + (mus @ mus)
        Gst = g[:B, B:] - np.add.outer(Xs @ mut, Xt @ mus) + (mus @ mut)
        Gtt = g[B:, B:] - np.add.outer(Xt @ mut, Xt @ mut) + (mut @ mut)
        Gss_sum += Gss
        Gst_sum += Gst
        Gtt_sum += Gtt
        num = (Gss * Gss).sum() - 2.0 * (Gst * Gst).sum() + (Gtt * Gtt).sum()
        W[t] = num / (Bm1 * Bm1 * 4.0 * FW * FW)

        # C matrices: out_c[a, (src, n, b)] = C_src[n, a, b] (bf16)
        c = r["out_c"].astype(np.float64).reshape(32, 2, N, D)
        Cs = c[:, 0].transpose(1, 0, 2) / Bm1   # [n, a, b]
        Ct = c[:, 1].transpose(1, 0, 2) / Bm1
        ss = np.einsum("nab,nab->n", Cs, Cs)
        tt = np.einsum("nab,nab->n", Ct, Ct)
        st = np.einsum("nab,jab->nj", Cs, Ct)
        Dm = (ss[:, None] + tt[None, :] - 2.0 * st) / (4.0 * D * D)
        pos = np.diag(Dm)
        neg = Dm.sum(axis=1) - pos
        L_sfa[t] = np.mean(np.log(np.exp(pos) + neg + EPS) - pos)

        e = r["out_e"].astype(np.float64).reshape(128, 2, ECH * 2)
        sums = e[:, 0, :].reshape(128, ECH, 2)
        sumsq = e[:, 1, :].reshape(128, ECH, 2)
        var = (sumsq - sums * sums / B) / Bm1
        dv = var[:, :, 0] - var[:, :, 1]
        L_sca[t] = np.mean(dv * dv) / 4.0

    fexo = T * FW
    num = ((Gss_sum * Gss_sum).sum() - 2.0 * (Gst_sum * Gst_sum).sum()
           + (Gtt_sum * Gtt_sum).sum())
    L_exo = num / (Bm1 * Bm1 * 4.0 * fexo * fexo)
    L_iendo = float((W * (LAM * L_sca + LAM * L_sfa)).sum())
    return np.float32(L_exo + L_iendo / T)


def _run(Z_s, E_s, Z_t, E_t, trace=False, **kw):
    nc = _get_nc()
    in_maps = _prep_in_maps(Z_s, E_s, Z_t, E_t)
    res = run_bass_kernel_spmd(nc, in_maps, core_ids=list(range(T)),
                               trace=trace, **kw)
    return _combine(res.results, Z_s, Z_t), res


def kernel(Z_s, E_s, Z_t, E_t):
    out, _ = _run(Z_s, E_s, Z_t, E_t)
    return out


# revision 20
# speedup vs baseline: 1.0830x; 1.0492x over previous
"""Trainium2 Bass kernel for nn_Alignment loss (CORAL-style alignment loss).

Strategy (hardcoded for B=64, hat_L=8, N=16, d=32, 8 cores):
  - Shard over hat_L: core i handles layer t=i (SPMD, per-core input shards).
  - All covariance Frobenius terms use the Gram trick:
      ||Xc^T Xc - Yc^T Yc||_F^2 = ||Xc Xc^T||^2 - 2||Xc Yc^T||^2 + ||Yc Yc^T||^2
    so the device only materializes 64x64 batch Grams, never feature covs.
  - The batch Gram is computed on RAW (uncentered) data in exact fp32; the
    rank-1 centering correction is applied on host in float64 from the raw
    inputs.  L_exo Grams are sums of per-t Grams (feature blocks).
  - The L_sfa tail (centering + transpose + per-node covariances) runs in
    bf16: its final-loss contribution is ~1%, so bf16 error is ~1e-5 on the
    output.  The 16x16 covariance inner products are done on host in
    float64 from the shipped bf16 C matrices.
  - E variance statistics use PE ones-matmuls on batch-major data; the
    ones column is embedded in the input/scratch images so every PE matmul
    carries at most one semaphore wait (hardware limit).
  - Inputs are host-packed into exact SBUF images (contiguous per-partition
    runs -> minimal DMA descriptor cost), one DMA per queue.
  - Device outputs per core: raw 2x2 block Gram [128,128] f32, bf16
    per-node covariances [32, 2*16*32], and E-sum/E-sumsq [128,8] f32.
"""

import numpy as np

import concourse.bass as bass
import concourse.tile as tile
from concourse import mybir
from concourse.bass_utils import run_bass_kernel_spmd

B = 64
T = 8
N = 16
D = 32
FW = N * D          # 512 flattened per-layer features
KCH = FW // 128     # 4 feature chunks of 128
ECH = (N * N) // 128  # 2 chunks for E features (256)
F32 = mybir.dt.float32
BF16 = mybir.dt.bfloat16

_BUILT = None


def _build():
    nc = bass.Bass()
    # za: chunks 0-1, zb: chunks 2-3; each [128, (k2, s, b)] image
    za = nc.dram_tensor("za", [128, 2 * 2 * B], F32, kind="ExternalInput")
    zb = nc.dram_tensor("zb", [128, 2 * 2 * B], F32, kind="ExternalInput")
    ee = nc.dram_tensor("ee", [B, 2 * N * N + 1], F32, kind="ExternalInput")
    out_g = nc.dram_tensor("out_g", [128, 128], F32, kind="ExternalOutput")
    out_c = nc.dram_tensor("out_c", [32, 2 * N * D], BF16,
                           kind="ExternalOutput")
    out_e = nc.dram_tensor("out_e", [128, 2, ECH * 2], F32,
                           kind="ExternalOutput")

    with tile.TileContext(nc) as tc:
        with tc.tile_pool(name="sb", bufs=1) as sb, \
             tc.tile_pool(name="ps1", bufs=1, space="PSUM") as ps1:
            # ---- loads: one packed image per DMA queue -------------------
            Zb = sb.tile([128, KCH, 2, B], F32)   # interleaved [Zs_k|Zt_k]
            Ebm = sb.tile([B, 2 * N * N + 1], F32)  # batch-major E + ones
            nc.sync.dma_start(
                out=Zb[:, 0:2, :, :],
                in_=za[:].rearrange("p (k s b) -> p k s b", s=2, b=B))
            nc.scalar.dma_start(
                out=Zb[:, 2:4, :, :],
                in_=zb[:].rearrange("p (k s b) -> p k s b", s=2, b=B))
            nc.gpsimd.dma_start(out=Ebm[:, :], in_=ee[:])

            # identity built on Pool, then fenced through the DVE so the
            # transposes wait on a single (DVE) semaphore
            identity0 = sb.tile([128, 128], BF16)
            nc.gpsimd.memset(identity0[:, :], 0.0)
            nc.gpsimd.affine_select(
                out=identity0[:, :], in_=identity0[:, :],
                compare_op=mybir.AluOpType.not_equal, fill=1.0,
                base=0, pattern=[[-1, 128]], channel_multiplier=1)
            identity = sb.tile([128, 128], BF16)
            nc.vector.tensor_copy(out=identity[:, :], in_=identity0[:, :])
            # warm the ACT table for Copy while DMAs are in flight
            warm = sb.tile([1, 1], F32)
            nc.vector.memset(warm[:, :], 0.0)
            nc.scalar.copy(out=warm[:, :], in_=warm[:, :])

            # ---- center Z over batch -> bf16, per chunk-pair on DVE ------
            zsums = sb.tile([128, 2, 2, 2], F32)  # [p, pair, s, k2]
            Zc = sb.tile([128, KCH, 2, B], BF16)
            for pair in range(2):
                ks = slice(2 * pair, 2 * pair + 2)
                for s in range(2):
                    nc.vector.reduce_sum(out=zsums[:, pair, s, :],
                                         in_=Zb[:, ks, s, :],
                                         axis=mybir.AxisListType.X)
                    sums_b = zsums[:, pair, s, :].broadcast_to([128, 2, B])
                    nc.vector.scalar_tensor_tensor(
                        out=Zc[:, ks, s, :], in0=sums_b, scalar=-1.0 / B,
                        in1=Zb[:, ks, s, :], op0=mybir.AluOpType.mult,
                        op1=mybir.AluOpType.add)

            # ---- raw 2x2 block batch Gram [128,128] (fp32, exact) --------
            gpsum = ps1.tile([128, 128], F32)
            for k in range(KCH):
                blk = Zb[:, k, :, :].rearrange("p s b -> p (s b)")
                nc.tensor.matmul(gpsum[:, :], blk, blk,
                                 start=(k == 0), stop=(k == KCH - 1))
            Gsb = sb.tile([128, 128], F32)

            # ---- transpose centered Z (bf16) to batch-major --------------
            # Zbm rows: 0-63 = Zsc [64, 512], 64-127 = Ztc [64, 512]
            Zbm = sb.tile([128, KCH, 128], BF16)
            for half in range(2):
                tp = ps1.tile([128, 2, 128], BF16, tag=f"tp{half}")
                for i in range(2):
                    k = half * 2 + i
                    blk = Zc[:, k, :, :].rearrange("p s b -> p (s b)")
                    nc.tensor.transpose(tp[:, i, :], blk, identity[:, :])
                dst = Zbm[:, 2 * half:2 * half + 2, :]
                if half == 0:
                    nc.vector.tensor_copy(out=dst, in_=tp[:, :, :])
                else:
                    nc.scalar.copy(out=dst, in_=tp[:, :, :])

            # ---- per-node covariances C[n] = Zc_n^T Zc_n [32,32] ---------
            cst0 = ps1.tile([32, N, D], F32)
            cst1 = ps1.tile([32, N, D], F32)
            STcat0 = sb.tile([32, N, D], BF16)
            STcat1 = sb.tile([32, N, D], BF16)
            for src in range(2):
                lo, hi = (0, B) if src == 0 else (B, 128)
                cst = cst0 if src == 0 else cst1
                for n in range(N):
                    k, c0 = divmod(n * D, 128)
                    lhs = Zbm[lo:hi, k, c0:c0 + D]
                    nc.tensor.matmul(cst[:, n, :], lhs, lhs,
                                     start=True, stop=True)
                # per-source copy (distinct PSUM tiles -> concurrent)
                if src == 0:
                    nc.vector.tensor_copy(out=STcat0[:, :, :],
                                          in_=cst0[:, :, :])
                else:
                    nc.scalar.copy(out=STcat1[:, :, :], in_=cst1[:, :, :])
            nc.sync.dma_start(
                out=out_c[:, 0:N * D],
                in_=STcat0[:, :, :].rearrange("p n b -> p (n b)"))
            nc.sync.dma_start(
                out=out_c[:, N * D:],
                in_=STcat1[:, :, :].rearrange("p n b -> p (n b)"))

            nc.vector.tensor_copy(out=Gsb[:, :], in_=gpsum[:, :])
            nc.sync.dma_start(out=out_g[:, :], in_=Gsb[:, :])

            # ---- E sums / sumsq via PE ones-matmuls (off the DVE) --------
            # Esq carries its own ones column so the sumsq matmuls wait on
            # the Pool sem only; the sums matmuls wait on the ee DMA only.
            Esq = sb.tile([B, 2 * N * N + 1], F32)
            nc.gpsimd.tensor_mul(Esq[:, 0:2 * N * N],
                                 Ebm[:, 0:2 * N * N], Ebm[:, 0:2 * N * N])
            nc.gpsimd.memset(Esq[:, 2 * N * N:], 1.0)
            epsum = ps1.tile([128, 2, ECH * 2], F32)
            ev = Ebm[:, 0:2 * N * N].rearrange("p (s f) -> p s f", s=2)
            qv = Esq[:, 0:2 * N * N].rearrange("p (s f) -> p s f", s=2)
            for s in range(2):
                for c in range(ECH):
                    nc.tensor.matmul(
                        epsum[:, 0, 2 * c + s:2 * c + s + 1],
                        ev[:, s, 128 * c:128 * (c + 1)],
                        Ebm[:, 2 * N * N:], start=True, stop=True)
                    nc.tensor.matmul(
                        epsum[:, 1, 2 * c + s:2 * c + s + 1],
                        qv[:, s, 128 * c:128 * (c + 1)],
                        Esq[:, 2 * N * N:], start=True, stop=True)
            ES = sb.tile([128, 2, ECH * 2], F32)
            nc.scalar.copy(out=ES[:, :, :], in_=epsum[:, :, :])
            nc.gpsimd.dma_start(out=out_e[:, :, :], in_=ES[:, :, :])

    return nc


def _get_nc():
    global _BUILT
    if _BUILT is None:
        _BUILT = _build()
    return _BUILT


def _prep_in_maps(Z_s, E_s, Z_t, E_t):
    in_maps = []
    for t in range(T):
        # Zb image: [128 p, k, s, b] = Z_src[b, 128k+p], split by chunk pair
        zzi = np.empty((128, KCH, 2, B), np.float32)
        zzi[:, :, 0, :] = Z_s[:, t].reshape(B, KCH, 128).transpose(2, 1, 0)
        zzi[:, :, 1, :] = Z_t[:, t].reshape(B, KCH, 128).transpose(2, 1, 0)
        # E image: batch-major [B, 2*256], plus a trailing ones column
        eei = np.empty((B, 2 * N * N + 1), np.float32)
        eei[:, 0:N * N] = E_s[:, t].reshape(B, N * N)
        eei[:, N * N:2 * N * N] = E_t[:, t].reshape(B, N * N)
        eei[:, 2 * N * N] = 1.0
        in_maps.append({
            "za": np.ascontiguousarray(zzi[:, 0:2].reshape(128, 2 * 2 * B)),
            "zb": np.ascontiguousarray(zzi[:, 2:4].reshape(128, 2 * 2 * B)),
            "ee": np.ascontiguousarray(eei),
        })
    return in_maps


def _combine(results, Z_s, Z_t):
    """Host-side (float64) combine of per-core partial reductions."""
    LAM = 0.1
    EPS = 1e-8
    Bm1 = B - 1

    Gss_sum = np.zeros((B, B), np.float64)
    Gst_sum = np.zeros((B, B), np.float64)
    Gtt_sum = np.zeros((B, B), np.float64)
    W = np.zeros(T, np.float64)
    L_sca = np.zeros(T, np.float64)
    L_sfa = np.zeros(T, np.float64)

    for t in range(T):
        r = results[t]
        g = r["out_g"].astype(np.float64).reshape(128, 128)
        # exact rank-1 centering corrections from the raw inputs
        Xs = Z_s[:, t].reshape(B, FW).astype(np.float64)
        Xt = Z_t[:, t].reshape(B, FW).astype(np.float64)
        mus, mut = Xs.mean(0), Xt.mean(0)
        Gss = g[:B, :B] - np.add.outer(Xs @ mus, Xs @ mus) + (mus @ mus)
        Gst = g[:B, B:] - np.add.outer(Xs @ mut, Xt @ mus) + (mus @ mut)
        Gtt = g[B:, B:] - np.add.outer(Xt @ mut, Xt @ mut) + (mut @ mut)
        Gss_sum += Gss
        Gst_sum += Gst
        Gtt_sum += Gtt
        num = (Gss * Gss).sum() - 2.0 * (Gst * Gst).sum() + (Gtt * Gtt).sum()
        W[t] = num / (Bm1 * Bm1 * 4.0 * FW * FW)

        # C matrices: out_c[a, (src, n, b)] = C_src[n, a, b] (bf16)
        c = r["out_c"].astype(np.float64).reshape(32, 2, N, D)
        Cs = c[:, 0].transpose(1, 0, 2) / Bm1   # [n, a, b]
        Ct = c[:, 1].transpose(1, 0, 2) / Bm1
        ss = np.einsum("nab,nab->n", Cs, Cs)
        tt = np.einsum("nab,nab->n", Ct, Ct)
        st = np.einsum("nab,jab->nj", Cs, Ct)
        Dm = (ss[:, None] + tt[None, :] - 2.0 * st) / (4.0 * D * D)
        pos = np.diag(Dm)
        neg = Dm.sum(axis=1) - pos
        L_sfa[t] = np.mean(np.log(np.exp(pos) + neg + EPS) - pos)

        e = r["out_e"].astype(np.float64).reshape(128, 2, ECH * 2)
        sums = e[:, 0, :].reshape(128, ECH, 2)
        sumsq = e[:, 1, :].reshape(128, ECH, 2)
        var = (sumsq - sums * sums / B) / Bm1
        dv = var[:, :, 0] - var[:, :, 1]
        L_sca[t] = np.mean(dv * dv) / 4.0

    fexo = T * FW
    num = ((Gss_sum * Gss_sum).sum() - 2.0 * (Gst_sum * Gst_sum).sum()
           + (Gtt_sum * Gtt_sum).sum())
    L_exo = num / (Bm1 * Bm1 * 4.0 * fexo * fexo)
    L_iendo = float((W * (LAM * L_sca + LAM * L_sfa)).sum())
    return np.float32(L_exo + L_iendo / T)


def _run(Z_s, E_s, Z_t, E_t, trace=False, **kw):
    nc = _get_nc()
    in_maps = _prep_in_maps(Z_s, E_s, Z_t, E_t)
    res = run_bass_kernel_spmd(nc, in_maps, core_ids=list(range(T)),
                               trace=trace, **kw)
    return _combine(res.results, Z_s, Z_t), res


def kernel(Z_s, E_s, Z_t, E_t):
    out, _ = _run(Z_s, E_s, Z_t, E_t)
    return out


# revision 21
# speedup vs baseline: 1.1409x; 1.0535x over previous
"""Trainium2 Bass kernel for nn_Alignment loss (CORAL-style alignment loss).

Strategy (hardcoded for B=64, hat_L=8, N=16, d=32, 8 cores):
  - Shard over hat_L: core i handles layer t=i (SPMD, per-core input shards).
  - All covariance Frobenius terms use the Gram trick:
      ||Xc^T Xc - Yc^T Yc||_F^2 = ||Xc Xc^T||^2 - 2||Xc Yc^T||^2 + ||Yc Yc^T||^2
    so the device only materializes 64x64 batch Grams, never feature covs.
  - The batch Gram is computed on RAW (uncentered) data in exact fp32; the
    rank-1 centering correction is applied on host in float64 from the raw
    inputs.  L_exo Grams are sums of per-t Grams (feature blocks).
  - The L_sfa tail (centering + transpose + per-node covariances) runs in
    bf16: its final-loss contribution is ~1%, so bf16 error is ~1e-5 on the
    output.  The 16x16 covariance inner products are done on host in
    float64 from the shipped bf16 C matrices.
  - E variance statistics use PE ones-matmuls on batch-major data; the
    ones column is embedded in the input/scratch images so every PE matmul
    carries at most one semaphore wait (hardware limit).
  - Inputs are host-packed into exact SBUF images (contiguous per-partition
    runs -> minimal DMA descriptor cost), one DMA per queue.
  - Device outputs per core: raw 2x2 block Gram [128,128] f32, bf16
    per-node covariances [32, 2*16*32], and E-sum/E-sumsq [128,8] f32.
"""

import numpy as np

import concourse.bass as bass
import concourse.tile as tile
from concourse import mybir
from concourse.bass_utils import run_bass_kernel_spmd

B = 64
T = 8
N = 16
D = 32
FW = N * D          # 512 flattened per-layer features
KCH = FW // 128     # 4 feature chunks of 128
ECH = (N * N) // 128  # 2 chunks for E features (256)
F32 = mybir.dt.float32
BF16 = mybir.dt.bfloat16

_BUILT = None


def _build():
    nc = bass.Bass()
    zz = nc.dram_tensor("zz", [128, KCH * 2 * B], F32, kind="ExternalInput")
    ee = nc.dram_tensor("ee", [B, 2 * N * N + 1], F32, kind="ExternalInput")
    out_o0 = nc.dram_tensor("out_o0", [128, 392], F32, kind="ExternalOutput")
    out_c1 = nc.dram_tensor("out_c1", [32, N * D], BF16,
                            kind="ExternalOutput")

    with tile.TileContext(nc) as tc:
        with tc.tile_pool(name="sb", bufs=1) as sb, \
             tc.tile_pool(name="ps1", bufs=1, space="PSUM") as ps1:
            # ---- loads: one packed image per DMA queue -------------------
            Zb = sb.tile([128, KCH, 2, B], F32)   # interleaved [Zs_k|Zt_k]
            Ebm = sb.tile([B, 2 * N * N + 1], F32)  # batch-major E + ones
            nc.sync.dma_start(
                out=Zb[:, :, :, :],
                in_=zz[:].rearrange("p (k s b) -> p k s b", s=2, b=B))
            nc.gpsimd.dma_start(out=Ebm[:, :], in_=ee[:])

            # identity built on Pool, then fenced through the DVE so the
            # transposes wait on a single (DVE) semaphore
            identity0 = sb.tile([128, 128], BF16)
            nc.gpsimd.memset(identity0[:, :], 0.0)
            nc.gpsimd.affine_select(
                out=identity0[:, :], in_=identity0[:, :],
                compare_op=mybir.AluOpType.not_equal, fill=1.0,
                base=0, pattern=[[-1, 128]], channel_multiplier=1)
            identity = sb.tile([128, 128], BF16)
            nc.vector.tensor_copy(out=identity[:, :], in_=identity0[:, :])
            # warm the ACT table for Copy while DMAs are in flight
            warm = sb.tile([1, 1], F32)
            nc.vector.memset(warm[:, :], 0.0)
            nc.scalar.copy(out=warm[:, :], in_=warm[:, :])

            # ---- center Z over batch -> bf16, per chunk-pair on DVE ------
            zsums = sb.tile([128, 2, 2, 2], F32)  # [p, pair, s, k2]
            Zc = sb.tile([128, KCH, 2, B], BF16)
            for pair in range(2):
                ks = slice(2 * pair, 2 * pair + 2)
                for s in range(2):
                    nc.vector.reduce_sum(out=zsums[:, pair, s, :],
                                         in_=Zb[:, ks, s, :],
                                         axis=mybir.AxisListType.X)
                    sums_b = zsums[:, pair, s, :].broadcast_to([128, 2, B])
                    nc.vector.scalar_tensor_tensor(
                        out=Zc[:, ks, s, :], in0=sums_b, scalar=-1.0 / B,
                        in1=Zb[:, ks, s, :], op0=mybir.AluOpType.mult,
                        op1=mybir.AluOpType.add)

            # ---- raw 2x2 block batch Gram [128,128] (fp32, exact) --------
            gpsum = ps1.tile([128, 128], F32)
            for k in range(KCH):
                blk = Zb[:, k, :, :].rearrange("p s b -> p (s b)")
                nc.tensor.matmul(gpsum[:, :], blk, blk,
                                 start=(k == 0), stop=(k == KCH - 1))

            # ---- transpose centered Z (bf16) to batch-major --------------
            # Zbm rows: 0-63 = Zsc [64, 512], 64-127 = Ztc [64, 512]
            Zbm = sb.tile([128, KCH, 128], BF16)
            for half in range(2):
                tp = ps1.tile([128, 2, 128], BF16, tag=f"tp{half}")
                for i in range(2):
                    k = half * 2 + i
                    blk = Zc[:, k, :, :].rearrange("p s b -> p (s b)")
                    nc.tensor.transpose(tp[:, i, :], blk, identity[:, :])
                dst = Zbm[:, 2 * half:2 * half + 2, :]
                if half == 0:
                    nc.vector.tensor_copy(out=dst, in_=tp[:, :, :])
                else:
                    nc.scalar.copy(out=dst, in_=tp[:, :, :])

            # ---- per-node covariances C[n] = Zc_n^T Zc_n [32,32] ---------
            cst0 = ps1.tile([32, N, D], F32)
            cst1 = ps1.tile([32, N, D], F32)
            O0 = sb.tile([128, 392], F32)
            nc.vector.memset(O0[:, 136:392], 0.0)
            STcat0 = O0[0:32, 136:392].bitcast(BF16).rearrange(
                "p (n b) -> p n b", n=N)
            STcat1 = sb.tile([32, N, D], BF16)
            for src in range(2):
                lo, hi = (0, B) if src == 0 else (B, 128)
                cst = cst0 if src == 0 else cst1
                for n in range(N):
                    k, c0 = divmod(n * D, 128)
                    lhs = Zbm[lo:hi, k, c0:c0 + D]
                    nc.tensor.matmul(cst[:, n, :], lhs, lhs,
                                     start=True, stop=True)
                # per-source copy (distinct PSUM tiles -> concurrent)
                if src == 0:
                    nc.vector.tensor_copy(out=STcat0[:, :, :],
                                          in_=cst0[:, :, :])
                else:
                    nc.scalar.copy(out=STcat1[:, :, :], in_=cst1[:, :, :])
            nc.scalar.dma_start(
                out=out_c1[:, :],
                in_=STcat1[:, :, :].rearrange("p n b -> p (n b)"))

            nc.vector.tensor_copy(out=O0[:, 0:128], in_=gpsum[:, :])

            # ---- E sums / sumsq via PE ones-matmuls (off the DVE) --------
            # Esq carries its own ones column so the sumsq matmuls wait on
            # the Pool sem only; the sums matmuls wait on the ee DMA only.
            Esq = sb.tile([B, 2 * N * N + 1], F32)
            nc.gpsimd.tensor_mul(Esq[:, 0:2 * N * N],
                                 Ebm[:, 0:2 * N * N], Ebm[:, 0:2 * N * N])
            nc.gpsimd.memset(Esq[:, 2 * N * N:], 1.0)
            epsum = ps1.tile([128, 2, ECH * 2], F32)
            ev = Ebm[:, 0:2 * N * N].rearrange("p (s f) -> p s f", s=2)
            qv = Esq[:, 0:2 * N * N].rearrange("p (s f) -> p s f", s=2)
            for s in range(2):
                for c in range(ECH):
                    nc.tensor.matmul(
                        epsum[:, 0, 2 * c + s:2 * c + s + 1],
                        ev[:, s, 128 * c:128 * (c + 1)],
                        Ebm[:, 2 * N * N:], start=True, stop=True)
                    nc.tensor.matmul(
                        epsum[:, 1, 2 * c + s:2 * c + s + 1],
                        qv[:, s, 128 * c:128 * (c + 1)],
                        Esq[:, 2 * N * N:], start=True, stop=True)
            nc.vector.tensor_copy(
                out=O0[:, 128:136].rearrange("p (u v) -> p u v", u=2),
                in_=epsum[:, :, :])
            nc.sync.dma_start(out=out_o0[:, :], in_=O0[:, :])

    return nc


def _get_nc():
    global _BUILT
    if _BUILT is None:
        _BUILT = _build()
    return _BUILT


def _prep_in_maps(Z_s, E_s, Z_t, E_t):
    in_maps = []
    for t in range(T):
        # Zb image: [128 p, k, s, b] = Z_src[b, 128k+p], split by chunk pair
        zzi = np.empty((128, KCH, 2, B), np.float32)
        zzi[:, :, 0, :] = Z_s[:, t].reshape(B, KCH, 128).transpose(2, 1, 0)
        zzi[:, :, 1, :] = Z_t[:, t].reshape(B, KCH, 128).transpose(2, 1, 0)
        # E image: batch-major [B, 2*256], plus a trailing ones column
        eei = np.empty((B, 2 * N * N + 1), np.float32)
        eei[:, 0:N * N] = E_s[:, t].reshape(B, N * N)
        eei[:, N * N:2 * N * N] = E_t[:, t].reshape(B, N * N)
        eei[:, 2 * N * N] = 1.0
        in_maps.append({
            "zz": np.ascontiguousarray(zzi.reshape(128, KCH * 2 * B)),
            "ee": np.ascontiguousarray(eei),
        })
    return in_maps


def _combine(results, Z_s, Z_t):
    """Host-side (float64) combine of per-core partial reductions."""
    LAM = 0.1
    EPS = 1e-8
    Bm1 = B - 1

    Gss_sum = np.zeros((B, B), np.float64)
    Gst_sum = np.zeros((B, B), np.float64)
    Gtt_sum = np.zeros((B, B), np.float64)
    W = np.zeros(T, np.float64)
    L_sca = np.zeros(T, np.float64)
    L_sfa = np.zeros(T, np.float64)

    for t in range(T):
        r = results[t]
        o0 = np.ascontiguousarray(r["out_o0"].reshape(128, 392))
        g = o0[:, 0:128].astype(np.float64)
        # exact rank-1 centering corrections from the raw inputs
        Xs = Z_s[:, t].reshape(B, FW).astype(np.float64)
        Xt = Z_t[:, t].reshape(B, FW).astype(np.float64)
        mus, mut = Xs.mean(0), Xt.mean(0)
        Gss = g[:B, :B] - np.add.outer(Xs @ mus, Xs @ mus) + (mus @ mus)
        Gst = g[:B, B:] - np.add.outer(Xs @ mut, Xt @ mus) + (mus @ mut)
        Gtt = g[B:, B:] - np.add.outer(Xt @ mut, Xt @ mut) + (mut @ mut)
        Gss_sum += Gss
        Gst_sum += Gst
        Gtt_sum += Gtt
        num = (Gss * Gss).sum() - 2.0 * (Gst * Gst).sum() + (Gtt * Gtt).sum()
        W[t] = num / (Bm1 * Bm1 * 4.0 * FW * FW)

        # C matrices: [a, (n, b)] = C_src[n, a, b] (bf16)
        import ml_dtypes
        c0 = o0[0:32, 136:392].view(ml_dtypes.bfloat16).astype(
            np.float64).reshape(32, N, D)
        c1 = np.asarray(r["out_c1"]).astype(np.float64).reshape(32, N, D)
        Cs = c0.transpose(1, 0, 2) / Bm1   # [n, a, b]
        Ct = c1.transpose(1, 0, 2) / Bm1
        ss = np.einsum("nab,nab->n", Cs, Cs)
        tt = np.einsum("nab,nab->n", Ct, Ct)
        st = np.einsum("nab,jab->nj", Cs, Ct)
        Dm = (ss[:, None] + tt[None, :] - 2.0 * st) / (4.0 * D * D)
        pos = np.diag(Dm)
        neg = Dm.sum(axis=1) - pos
        L_sfa[t] = np.mean(np.log(np.exp(pos) + neg + EPS) - pos)

        e = o0[:, 128:136].astype(np.float64).reshape(128, 2, ECH * 2)
        sums = e[:, 0, :].reshape(128, ECH, 2)
        sumsq = e[:, 1, :].reshape(128, ECH, 2)
        var = (sumsq - sums * sums / B) / Bm1
        dv = var[:, :, 0] - var[:, :, 1]
        L_sca[t] = np.mean(dv * dv) / 4.0

    fexo = T * FW
    num = ((Gss_sum * Gss_sum).sum() - 2.0 * (Gst_sum * Gst_sum).sum()
           + (Gtt_sum * Gtt_sum).sum())
    L_exo = num / (Bm1 * Bm1 * 4.0 * fexo * fexo)
    L_iendo = float((W * (LAM * L_sca + LAM * L_sfa)).sum())
    return np.float32(L_exo + L_iendo / T)


def _run(Z_s, E_s, Z_t, E_t, trace=False, **kw):
    nc = _get_nc()
    in_maps = _prep_in_maps(Z_s, E_s, Z_t, E_t)
    res = run_bass_kernel_spmd(nc, in_maps, core_ids=list(range(T)),
                               trace=trace, **kw)
    return _combine(res.results, Z_s, Z_t), res


def kernel(Z_s, E_s, Z_t, E_t):
    out, _ = _run(Z_s, E_s, Z_t, E_t)
    return out
